# revision 38
# baseline (speedup 1.0000x reference)
"""CWSA (channel-wise self-attention) layer for Trainium2, 8 NeuronCores.

Math (per batch b of 4):
    x_q = W_qk @ x[b]                  # [64, 4096]   (k == q, tied weights)
    x_v = W_v  @ x[b] + b_v            # [64, 4096]
    E   = x_q^T x_q / 8                # [4096, 4096] Gram matrix
    A   = softmax(E, axis=-1)          # rows sum to 1
    out = x_v @ A                      # [64, 4096]
Sharding: 8 cores = 4 batches x 2 halves of the n (row/contraction) axis;
softmax rows stay core-local, each core emits a partial out and the host
sums the two partials per batch.

The kernel is a single exp stream on the scalar engine -- the hard
bottleneck: 64 x [128,1024] chunks at ~1.11us each (~72us busy; exp is
ScalarE-only and PSUM limits chunks to 1024 since the AV accumulators
hold the other 8KB/partition). Everything else hides under it:

  * ramp: input x is column-chunked per 128-row half over both DMA rings
    (ring FIFO delivers low columns first at full rate); the stream opens
    with eight 512-wide sub-chunk exps of tiles 0-3 whose first halves
    depend only on the first q projection, so exp starts ~3us after the
    first 128KB lands; hi-column chunks of tiles 0-3 run ~8 positions
    later, hiding the hi-half DMA + projection latency.
  * rowsums (the softmax denominators) never touch the scalar queue:
    chunks 0-2 of each tile are folded 1024->512 on the otherwise-idle
    gpsimd and reduced on vector (~660ns), chunk 3 is a direct vector
    reduce; the chain rs4 -> 1/rs -> xvs = xv/rs runs at raised priority
    so it is never reordered behind bulk reduces.
  * PE: energy fills row-slot-pack two K=64 matmuls (q duplicated across
    partition halves); AV matmuls are deprioritized gap fillers, emitted
    one tile late and spread bank-by-bank across the next tile's chunk
    positions so the in-order PE never starves a fill behind an AV burst.
  * tail: the last exp carries its rowsum via accum_out, the last tile's
    AV runs 512-wide in bank order, and each PSUM bank is copied
    (scalar/vector alternating) and DMA'd out as soon as it closes;
    gpsimd gets no epilogue work so its slow end-of-kernel drain overlaps
    the output DMAs.

Measured 104.2-105.4us on hardware (run-to-run DVFS variance ~3%), vs
~107us for the first working version and a ~72us scalar-exp busy floor
(+ ~6.3us engine-boot, ~6us first-DMA latency, ~7us tail).
"""

import sys

sys.path.insert(0, "/opt/trn_rl_repo")

import numpy as np
import ml_dtypes

import concourse.bass as bass
import concourse.mybir as mybir
import concourse.tile as tile
from concourse import bacc
from concourse.bass import ts, ds

B = 4
C = 256
C4 = 64
N = 4096
NH = N // 2          # n rows per core
NT = 128             # n-tile rows
NTILES = NH // NT    # 16
FACTOR = float(np.sqrt(C4))  # 8.0

BF16 = mybir.dt.bfloat16
F32 = mybir.dt.float32
EXP = mybir.ActivationFunctionType.Exp
ADD = mybir.AluOpType.add
MULT = mybir.AluOpType.mult


def build_nc() -> bass.Bass:
    nc = bacc.Bacc("TRN2", target_bir_lowering=False, debug=False, num_devices=8)

    x_m = nc.declare_dram_parameter("x_m", [C, N], BF16, isOutput=False)
    wq_t = nc.declare_dram_parameter("wq_t", [C, C4], BF16, isOutput=False)
    wv_t = nc.declare_dram_parameter("wv_t", [C, C4], BF16, isOutput=False)
    bv = nc.declare_dram_parameter("bv", [C4], BF16, isOutput=False)
    out_p = nc.declare_dram_parameter("out_p", [C4, N], BF16, isOutput=True)

    from contextlib import ExitStack

    with tile.TileContext(nc) as tc, ExitStack() as ctx:
        sing = ctx.enter_context(tc.tile_pool(name="sing", bufs=1))
        small = ctx.enter_context(tc.tile_pool(name="small", bufs=6))
        # hf gets a deep ring of its own: the gpsimd folds must not WAR-wait
        # on vector's reduce backlog (vector drains casts early on).
        hfp = ctx.enter_context(tc.tile_pool(name="hfp", bufs=12))
        work = ctx.enter_context(tc.tile_pool(name="work", bufs=10))
        e_ps = ctx.enter_context(tc.tile_pool(name="e_ps", bufs=2, space="PSUM"))
        xr_ps = ctx.enter_context(tc.tile_pool(name="xr_ps", bufs=1, space="PSUM"))

        # ---- input loads -------------------------------------------------
        # The host rotates x[b] per core so the local n-half is always
        # columns 0:2048. Each x DMA covers BOTH 128-row halves of one
        # column chunk (3D access pattern), so a projection becomes ready
        # the moment its single chunk lands. Chunks are issued low-half
        # first on both rings; ring FIFO order gives the low half strict
        # SDMA priority over the hi half.
        xm_sb = sing.tile([128, 2, N], BF16)
        wq_sb = sing.tile([128, 2, C4], BF16)
        wv_sb = sing.tile([128, 2, C4], BF16)
        bv_bc = sing.tile([128, C4], BF16)

        def x_src(col0, w):
            ap = x_m[:]
            return bass.AP(
                tensor=ap.tensor,
                offset=col0,
                ap=[[N, 128], [N * 128, 2], [1, w]],
            )

        def w_src(w_t):
            ap = w_t[:]
            return bass.AP(
                tensor=ap.tensor,
                offset=0,
                ap=[[C4, 128], [C4 * 128, 2], [1, C4]],
            )

        # Both rings (sync=HWDGE, gpsimd=SWDGE) load the SAME column ranges
        # in the same order, split by 128-row half: ring FIFO then delivers
        # each column range at the aggregate rate, in exp-stream order.
        def x2(ch, a, b):
            return x_m[ts(ch, 128), a:b]

        # c0 rows on the fast HWDGE (sync) ring, c1 rows on the SWDGE
        # (gpsimd) ring, both in exp-stream column order: each ring's FIFO
        # delivers the low columns first at full rate.
        nc.sync.dma_start(out=xm_sb[:, 0, 0:512], in_=x2(0, 0, 512))
        nc.sync.dma_start(out=wq_sb, in_=w_src(wq_t))
        nc.sync.dma_start(out=xm_sb[:, 0, 512:1024], in_=x2(0, 512, 1024))
        nc.sync.dma_start(out=xm_sb[:, 0, 1024:2048], in_=x2(0, 1024, 2048))
        nc.sync.dma_start(out=xm_sb[:, 0, 2048:3072], in_=x2(0, 2048, 3072))
        nc.sync.dma_start(out=xm_sb[:, 0, 3072:4096], in_=x2(0, 3072, 4096))
        nc.gpsimd.dma_start(out=xm_sb[:, 1, 0:512], in_=x2(1, 0, 512))
        nc.gpsimd.dma_start(out=wv_sb, in_=w_src(wv_t))
        nc.gpsimd.dma_start(out=xm_sb[:, 1, 512:1024], in_=x2(1, 512, 1024))
        nc.gpsimd.dma_start(out=xm_sb[:, 1, 1024:2048], in_=x2(1, 1024, 2048))
        nc.gpsimd.dma_start(out=xm_sb[:, 1, 2048:3072], in_=x2(1, 2048, 3072))
        nc.gpsimd.dma_start(out=xm_sb[:, 1, 3072:4096], in_=x2(1, 3072, 4096))
        bv_ap = bv[:]
        bv_bcast = bass.AP(
            tensor=bv_ap.tensor, offset=bv_ap.offset, ap=[[0, 128]] + list(bv_ap.ap)
        )
        nc.gpsimd.dma_start(out=bv_bc, in_=bv_bcast)

        # ---- projections -------------------------------------------------
        # q is stored twice along partitions (0:64 and 64:128) so energy
        # fills can row-slot-pack two K=64 matmuls into the PE array.
        def colpack_proj(dst_ps, rhs0, rhs1):
            return [
                nc.tensor.matmul(dst_ps[0:64, :], wq_sb[:, 0, :], rhs0,
                                 start=True, stop=False, tile_position=(0, 0)),
                nc.tensor.matmul(dst_ps[64:128, :], wq_sb[:, 0, :], rhs0,
                                 start=True, stop=False, tile_position=(0, 64),
                                 skip_group_check=True),
                nc.tensor.matmul(dst_ps[0:64, :], wq_sb[:, 1, :], rhs1,
                                 start=False, stop=True, tile_position=(0, 0)),
                nc.tensor.matmul(dst_ps[64:128, :], wq_sb[:, 1, :], rhs1,
                                 start=False, stop=True, tile_position=(0, 64),
                                 skip_group_check=True),
            ]

        xqt = [sing.tile([128, 1024], BF16, name=f"xq{i}") for i in range(4)]

        def xk(row, t):
            i, off = (t * NT) // 1024, (t * NT) % 1024
            return xqt[i][row:row + 64, off:off + NT]

        def xq(row, col, w):
            i, cc = col // 1024, col % 1024
            return xqt[i][row:row + 64, cc:cc + w]

        def q_proj(j, prio=0):
            qp = xr_ps.tile([128, 512], F32, tag=f"xr{j % 4}", name=f"qp{j}")
            mms = colpack_proj(qp, xm_sb[:, 0, ts(j, 512)], xm_sb[:, 1, ts(j, 512)])
            for m in mms:
                m.ins.bass_priority = prio
            dst = xqt[j // 2][:, (j % 2) * 512:(j % 2) * 512 + 512]
            # all casts on vector: the scalar queue stays pure exp (any op
            # queued ahead of the first exp delays the whole stream).
            cp = nc.vector.tensor_copy(out=dst, in_=qp)
            cp.ins.bass_priority = -600

        # ---- energy fill / exp plumbing ----------------------------------
        # stream order: tiles 0/1 interleaved at the front so the hi-half
        # (cols 2048:4096) DMA + projection latency hides under low-half
        # exp chunks.
        # stream order: the first eight items are 512-wide sub-chunks of
        # (t, 0) for tiles 0-3 -- the 'a' halves depend ONLY on the first
        # q projection (cols 0:512), so the exp stream starts the moment
        # the first 128KB of x lands, while q1..q7 project underneath.
        chunk_list = [(0, 0, 'a'), (1, 0, 'a'), (2, 0, 'a'), (3, 0, 'a'),
                      (0, 0, 'b'), (1, 0, 'b'), (2, 0, 'b'), (3, 0, 'b'),
                      (0, 1, None), (1, 1, None), (2, 1, None), (3, 1, None),
                      (0, 2, None), (0, 3, None), (1, 2, None), (1, 3, None),
                      (2, 2, None), (2, 3, None), (3, 2, None), (3, 3, None)]
        for t in range(4, NTILES):
            chunk_list += [(t, 0, None), (t, 1, None),
                           (t, 2, None), (t, 3, None)]

        def emit_fill(t, c, sub=None, prio=0, borrow=None):
            m0 = 1024 * c
            if sub == 'a':
                if borrow is not None:
                    # early runway chunks use the (still idle) AV
                    # accumulator banks as extra PSUM so they skip the
                    # 2-slot e-ring WAR entirely.
                    e_t = xr_ps.tile([128, 512], F32, tag=borrow,
                                     name=f"e{t}_{c}a")
                else:
                    e_t = e_ps.tile([128, 512], F32, tag="e", name=f"e{t}_{c}a")
                m1 = nc.tensor.matmul(e_t, xk(0, t), xq(0, m0, 512),
                                      start=True, stop=True,
                                      tile_position=(0, 0))
                m1.ins.bass_priority = prio
                return e_t
            if sub == 'b':
                e_t = e_ps.tile([128, 512], F32, tag="e", name=f"e{t}_{c}b")
                m1 = nc.tensor.matmul(e_t, xk(64, t), xq(64, m0 + 512, 512),
                                      start=True, stop=True,
                                      tile_position=(64, 0),
                                      skip_group_check=True)
                m1.ins.bass_priority = prio
                return e_t
            e_t = e_ps.tile([128, 1024], F32, tag="e", name=f"e{t}_{c}")
            m1 = nc.tensor.matmul(e_t[:, 0:512], xk(0, t), xq(0, m0, 512),
                                  start=True, stop=True, tile_position=(0, 0))
            m2 = nc.tensor.matmul(e_t[:, 512:1024], xk(64, t),
                                  xq(64, m0 + 512, 512),
                                  start=True, stop=True, tile_position=(64, 0),
                                  skip_group_check=True)
            m1.ins.bass_priority = prio
            m2.ins.bass_priority = prio
            return e_t

        # prologue: projections and the first two fills, interleaved so
        # each fill is emitted as soon as its q columns exist.
        q_proj(0, prio=-3000)
        etiles = {(0, 0, 'a'): emit_fill(0, 0, 'a', prio=-2995)}
        q_proj(1, prio=-2990)
        etiles[(1, 0, 'a')] = emit_fill(1, 0, 'a', prio=-2985)
        q_proj(2, prio=-2970)
        q_proj(3, prio=-2960)
        q_proj(4, prio=-2930)
        q_proj(5, prio=-2920)
        q_proj(6, prio=-2910)
        q_proj(7, prio=-2900)

        # per-tile v projections (deprioritized PE gap filler)
        xvt_sb = [
            sing.tile([128, C4], BF16, name=f"xvt{t}") for t in range(NTILES)
        ]
        for t in range(NTILES):
            vp = xr_ps.tile([128, C4], F32, tag=f"xr{t % 4}", name=f"vp{t}")
            half = t // 8
            off = (t % 8) * NT
            mm1 = nc.tensor.matmul(vp, xm_sb[:, 0, ds(half * 1024 + off, NT)],
                                   wv_sb[:, 0, :], start=True, stop=False)
            mm2 = nc.tensor.matmul(vp, xm_sb[:, 1, ds(half * 1024 + off, NT)],
                                   wv_sb[:, 1, :], start=False, stop=True)
            mm1.ins.bass_priority = 500_000 + 2 * t
            mm2.ins.bass_priority = 500_000 + 2 * t + 1
            nc.vector.tensor_add(out=xvt_sb[t], in0=vp, in1=bv_bc)

        # ---- output accumulators (partition-packed: even m-chunk in
        # partitions 0-63, odd in 64-127) -----------------------------------
        xr = [
            xr_ps.tile([128, 512], F32, tag=f"xr{k}", name=f"xr{k}")
            for k in range(4)
        ]

        p_tiles = {}
        xvs_tiles = {}
        rs4_tiles = {}

        def chunk_rowsum(t, c):
            rs4 = rs4_tiles[t]
            p = p_tiles[t]
            last_tile = t == NTILES - 1
            if last_tile and c == 3:
                return  # rowsum came from the exp's accumulator
            if c == 3 or (last_tile and c == 2):
                # direct reduce right after the chunk's exp (off the scalar
                # queue; for the last tile it finishes under the final exp)
                r = nc.vector.tensor_reduce(out=rs4[:, c:c + 1],
                                            in_=p[:, ds(1024 * c, 1024)],
                                            axis=mybir.AxisListType.X, op=ADD)
                if last_tile:
                    r.ins.bass_priority = -500
            else:
                hf = hfp.tile([128, 512], BF16, tag="hf")
                nc.gpsimd.tensor_add(out=hf, in0=p[:, ds(1024 * c, 512)],
                                     in1=p[:, ds(1024 * c + 512, 512)])
                nc.vector.tensor_reduce(out=rs4[:, c:c + 1], in_=hf,
                                        axis=mybir.AxisListType.X, op=ADD)

        def do_exp(t, c, sub):
            p = p_tiles[t]
            e_t = etiles.pop((t, c, sub))
            if t not in rs4_tiles:
                rs4_tiles[t] = small.tile([128, 4], F32, tag="rs4", name=f"rs4_{t}")
            rs4 = rs4_tiles[t]
            last_tile = t == NTILES - 1
            if sub == 'a':
                nc.scalar.activation(out=p[:, ds(1024 * c, 512)],
                                     in_=e_t, func=EXP)
                return
            if sub == 'b':
                nc.scalar.activation(out=p[:, ds(1024 * c + 512, 512)],
                                     in_=e_t, func=EXP)
            elif last_tile and c == 3:
                # the very last exp carries its own rowsum accumulator so
                # the final normalization starts ~300ns after it instead of
                # a 1.2us vector-reduce later.
                nc.scalar.activation(out=p[:, ds(1024 * c, 1024)], in_=e_t,
                                     func=EXP, accum_out=rs4[:, 3:4])
            else:
                nc.scalar.activation(out=p[:, ds(1024 * c, 1024)], in_=e_t,
                                     func=EXP)
            chunk_rowsum(t, c)

        def rowsum_tile(t):
            rs4 = rs4_tiles.pop(t)
            rs = small.tile([128, 1], F32, tag="rs")
            r1 = nc.vector.tensor_reduce(out=rs, in_=rs4,
                                         axis=mybir.AxisListType.X, op=ADD)
            rr = small.tile([128, 1], F32, tag="rr")
            r2 = nc.vector.reciprocal(out=rr, in_=rs)
            xvs = small.tile([128, C4], BF16, tag="xvs")
            r3 = nc.vector.tensor_scalar_mul(out=xvs, in0=xvt_sb[t], scalar1=rr)
            # the normalization chain gates AV(t): never let the scheduler
            # slip a bulk reduce ahead of it on the vector queue.
            for r in (r1, r2, r3):
                r.ins.bass_priority = -500
            xvs_tiles[t] = xvs

        def emit_av_bank(t, k):
            # one bank's worth of AV (4 matmuls): emitted at four separate
            # stream positions so the in-order PE never sees an AV burst
            # longer than ~1us between energy fills.
            p = p_tiles[t]
            xvs = xvs_tiles[t]
            first = t == 0
            last = t == NTILES - 1
            # 512-wide everywhere: AV bursts are already bounded to one
            # bank by the spread emission, and half the matmul/LDWEIGHTS
            # count keeps the PE ahead of the exp stream in HAM-throttled
            # windows.
            av_w = 512
            for j in (2 * k, 2 * k + 1):
                po = (j % 2) * 64
                for s in range(512 // av_w):
                    mm = nc.tensor.matmul(
                        xr[k][po:po + 64, ds(s * av_w, av_w)], xvs,
                        p[:, ds(j * 512 + s * av_w, av_w)],
                        start=first, stop=last, tile_position=(0, po),
                        skip_group_check=True,
                    )
                    if not last:
                        mm.ins.bass_priority = 1_000_000 + t * 100 + j * 4 + s

        def emit_av(t):
            for k in range(4):
                emit_av_bank(t, k)
            xvs_tiles.pop(t)

        # ---- the stream --------------------------------------------------
        # AV(t) is emitted one tile late (at (t+1, 3)) so in the in-order
        # PE queue ALL of tile t+1's fills statically precede AV(t): a late
        # xvs(t) can then never stall the exp stream behind an AV group.
        for i, (t, c, sub) in enumerate(chunk_list):
            if t not in p_tiles:
                p_tiles[t] = work.tile([128, N], BF16, tag="p", name=f"p{t}")
            do_exp(t, c, sub)
            if i + 2 < len(chunk_list):
                nt_, nc_, ns_ = chunk_list[i + 2]
                if (nt_, nc_, ns_) not in etiles:
                    prio = -2950 + i * 5 if i < 8 else 0
                    etiles[(nt_, nc_, ns_)] = emit_fill(nt_, nc_, ns_,
                                                        prio=prio)
            if sub is None and t >= 4 and (t - 1) in xvs_tiles:
                emit_av_bank(t - 1, c)
                if c == 3:
                    xvs_tiles.pop(t - 1)
            if c == 3 and sub is None:
                rowsum_tile(t)
                if t < 4 and t >= 1 and (t - 1) in xvs_tiles:
                    emit_av(t - 1)
                if t == NTILES - 1:
                    emit_av(t)

        # ---- epilogue: per-bank staggered PSUM->SBUF copy + DMA ----------
        # bf16 partials: the host sums the two per-batch partials in fp32;
        # bf16 here halves the output DMA drain and is well inside the
        # error budget (adds ~0.3% to a 0.46% rel err vs 2% tolerance).
        out_sb = sing.tile([128, 4, 512], BF16)
        # copies first (in bank-closure order), then the DMA issues:
        # scalar's FIFO must not block a later copy behind an earlier
        # bank's DMA issues. The last two banks gate the kernel end, so
        # their copies are split column-wise across BOTH engines (~0.45us
        # instead of ~0.69us each).
        for k in range(4):
            if k < 2:
                eng = nc.scalar.copy if k == 0 else None
                if k == 0:
                    nc.scalar.copy(out=out_sb[:, k, :], in_=xr[k])
                else:
                    nc.vector.tensor_copy(out=out_sb[:, k, :], in_=xr[k])
            else:
                nc.scalar.copy(out=out_sb[:, k, 0:256], in_=xr[k][:, 0:256])
                nc.vector.tensor_copy(out=out_sb[:, k, 256:512],
                                      in_=xr[k][:, 256:512])
        # each bank's two 128KB halves go to different rings so both
        # rings start draining at bank 0's closure and finish together.
        for k in range(4):
            nc.sync.dma_start(out=out_p[:, ts(2 * k, 512)],
                              in_=out_sb[0:64, k, :])
            nc.scalar.dma_start(out=out_p[:, ts(2 * k + 1, 512)],
                                in_=out_sb[64:128, k, :])

    nc.compile()
    return nc


_NC_CACHE = None


def _get_nc():
    global _NC_CACHE
    if _NC_CACHE is None:
        _NC_CACHE = build_nc()
    return _NC_CACHE


def make_in_maps(x, W_qk, W_v, b_v):
    bf = ml_dtypes.bfloat16
    x = np.asarray(x, dtype=np.float32)
    W_qk = np.asarray(W_qk, dtype=np.float32)
    W_v = np.asarray(W_v, dtype=np.float32)
    b_v = np.asarray(b_v, dtype=np.float32)
    xbf = np.ascontiguousarray(x).astype(bf)
    wqt = np.ascontiguousarray((W_qk / np.sqrt(FACTOR)).T).astype(bf)
    wvt = np.ascontiguousarray(W_v.T).astype(bf)
    bvb = np.ascontiguousarray(b_v).astype(bf)
    in_maps = []
    for core in range(8):
        b, h = core // 2, core % 2
        xm = xbf[b] if h == 0 else np.ascontiguousarray(
            np.roll(xbf[b], -NH, axis=1))
        in_maps.append({
            "x_m": xm,
            "wq_t": wqt,
            "wv_t": wvt,
            "bv": bvb,
        })
    return in_maps


def kernel(x, W_qk, W_v, b_v, _trace=False):
    from concourse.bass_utils import run_bass_kernel_spmd

    nc = _get_nc()
    in_maps = make_in_maps(x, W_qk, W_v, b_v)
    res = run_bass_kernel_spmd(nc, in_maps, list(range(8)), trace=_trace)
    if _trace:
        print(f"HW exec time: {res.exec_time_ns} ns")
        print(f"mean exec time: {res.mean_exec_time_ns} ns")
    outs = [np.asarray(res.results[i]["out_p"], dtype=np.float32)
            for i in range(8)]
    out = np.stack([
        outs[2 * b] + np.roll(outs[2 * b + 1], NH, axis=1) for b in range(B)
    ])
    return out.astype(np.float32)


# revision 39
# speedup vs baseline: 1.1789x; 1.1789x over previous
"""CWSA (channel-wise self-attention) layer for Trainium2, 8 NeuronCores.

Math (per batch b of 4):
    x_q = W_qk @ x[b]                  # [64, 4096]   (k == q, tied weights)
    x_v = W_v  @ x[b] + b_v            # [64, 4096]
    E   = x_q^T x_q / 8                # [4096, 4096] Gram matrix
    A   = softmax(E, axis=-1)          # rows sum to 1
    out = x_v @ A                      # [64, 4096]
Sharding: 8 cores = 4 batches x 2 halves of the n (row/contraction) axis;
softmax rows stay core-local, each core emits a partial out and the host
sums the two partials per batch.

The kernel is a single exp stream on the scalar engine -- the hard
bottleneck: 64 x [128,1024] chunks at ~1.11us each (~72us busy; exp is
ScalarE-only and PSUM limits chunks to 1024 since the AV accumulators
hold the other 8KB/partition). Everything else hides under it:

  * ramp: input x is column-chunked per 128-row half over both DMA rings
    (ring FIFO delivers low columns first at full rate); the stream opens
    with eight 512-wide sub-chunk exps of tiles 0-3 whose first halves
    depend only on the first q projection, so exp starts ~3us after the
    first 128KB lands; hi-column chunks of tiles 0-3 run ~8 positions
    later, hiding the hi-half DMA + projection latency.
  * rowsums (the softmax denominators) never touch the scalar queue:
    chunks 0-2 of each tile are folded 1024->512 on the otherwise-idle
    gpsimd and reduced on vector (~660ns), chunk 3 is a direct vector
    reduce; the chain rs4 -> 1/rs -> xvs = xv/rs runs at raised priority
    so it is never reordered behind bulk reduces.
  * PE: energy fills row-slot-pack two K=64 matmuls (q duplicated across
    partition halves); AV matmuls are deprioritized gap fillers, emitted
    one tile late and spread bank-by-bank across the next tile's chunk
    positions so the in-order PE never starves a fill behind an AV burst.
  * tail: the last exp carries its rowsum via accum_out, the last tile's
    AV runs 512-wide in bank order, and each PSUM bank is copied
    (scalar/vector alternating) and DMA'd out as soon as it closes;
    gpsimd gets no epilogue work so its slow end-of-kernel drain overlaps
    the output DMAs.

Measured 104.2-105.4us on hardware (run-to-run DVFS variance ~3%), vs
~107us for the first working version and a ~72us scalar-exp busy floor
(+ ~6.3us engine-boot, ~6us first-DMA latency, ~7us tail).
"""

import sys

sys.path.insert(0, "/opt/trn_rl_repo")

import numpy as np
import ml_dtypes

import concourse.bass as bass
import concourse.mybir as mybir
import concourse.tile as tile
from concourse import bacc
from concourse.bass import ts, ds

B = 4
C = 256
C4 = 64
N = 4096
NH = N // 2          # n rows per core
NT = 128             # n-tile rows
NTILES = NH // NT    # 16
FACTOR = float(np.sqrt(C4))  # 8.0

BF16 = mybir.dt.bfloat16
F32 = mybir.dt.float32
EXP = mybir.ActivationFunctionType.Exp
ADD = mybir.AluOpType.add
MULT = mybir.AluOpType.mult


def build_nc() -> bass.Bass:
    nc = bacc.Bacc("TRN2", target_bir_lowering=False, debug=False, num_devices=8)

    x_m = nc.declare_dram_parameter("x_m", [C, N], BF16, isOutput=False)
    wq_t = nc.declare_dram_parameter("wq_t", [C, C4], BF16, isOutput=False)
    wv_t = nc.declare_dram_parameter("wv_t", [C, C4], BF16, isOutput=False)
    bv = nc.declare_dram_parameter("bv", [C4], BF16, isOutput=False)
    out_p = nc.declare_dram_parameter("out_p", [C4, N], BF16, isOutput=True)

    from contextlib import ExitStack

    with tile.TileContext(nc) as tc, ExitStack() as ctx:
        sing = ctx.enter_context(tc.tile_pool(name="sing", bufs=1))
        small = ctx.enter_context(tc.tile_pool(name="small", bufs=6))
        # hf gets a deep ring of its own: the gpsimd folds must not WAR-wait
        # on vector's reduce backlog (vector drains casts early on).
        hfp = ctx.enter_context(tc.tile_pool(name="hfp", bufs=12))
        work = ctx.enter_context(tc.tile_pool(name="work", bufs=10))
        e_ps = ctx.enter_context(tc.tile_pool(name="e_ps", bufs=2, space="PSUM"))
        xr_ps = ctx.enter_context(tc.tile_pool(name="xr_ps", bufs=1, space="PSUM"))

        # ---- input loads -------------------------------------------------
        # The host rotates x[b] per core so the local n-half is always
        # columns 0:2048. Each x DMA covers BOTH 128-row halves of one
        # column chunk (3D access pattern), so a projection becomes ready
        # the moment its single chunk lands. Chunks are issued low-half
        # first on both rings; ring FIFO order gives the low half strict
        # SDMA priority over the hi half.
        xm_sb = sing.tile([128, 2, N], BF16)
        wq_sb = sing.tile([128, 2, C4], BF16)
        wv_sb = sing.tile([128, 2, C4], BF16)
        bv_bc = sing.tile([128, C4], BF16)

        def x_src(col0, w):
            ap = x_m[:]
            return bass.AP(
                tensor=ap.tensor,
                offset=col0,
                ap=[[N, 128], [N * 128, 2], [1, w]],
            )

        def w_src(w_t):
            ap = w_t[:]
            return bass.AP(
                tensor=ap.tensor,
                offset=0,
                ap=[[C4, 128], [C4 * 128, 2], [1, C4]],
            )

        # Both rings (sync=HWDGE, gpsimd=SWDGE) load the SAME column ranges
        # in the same order, split by 128-row half: ring FIFO then delivers
        # each column range at the aggregate rate, in exp-stream order.
        def x2(ch, a, b):
            return x_m[ts(ch, 128), a:b]

        # c0 rows on the fast HWDGE (sync) ring, c1 rows on the SWDGE
        # (gpsimd) ring, both in exp-stream column order: each ring's FIFO
        # delivers the low columns first at full rate.
        nc.sync.dma_start(out=xm_sb[:, 0, 0:512], in_=x2(0, 0, 512))
        nc.sync.dma_start(out=wq_sb, in_=w_src(wq_t))
        nc.sync.dma_start(out=xm_sb[:, 0, 512:1024], in_=x2(0, 512, 1024))
        nc.sync.dma_start(out=xm_sb[:, 0, 1024:2048], in_=x2(0, 1024, 2048))
        nc.sync.dma_start(out=xm_sb[:, 0, 2048:3072], in_=x2(0, 2048, 3072))
        nc.sync.dma_start(out=xm_sb[:, 0, 3072:4096], in_=x2(0, 3072, 4096))
        nc.gpsimd.dma_start(out=xm_sb[:, 1, 0:512], in_=x2(1, 0, 512))
        nc.gpsimd.dma_start(out=wv_sb, in_=w_src(wv_t))
        nc.gpsimd.dma_start(out=xm_sb[:, 1, 512:1024], in_=x2(1, 512, 1024))
        nc.gpsimd.dma_start(out=xm_sb[:, 1, 1024:2048], in_=x2(1, 1024, 2048))
        nc.gpsimd.dma_start(out=xm_sb[:, 1, 2048:3072], in_=x2(1, 2048, 3072))
        nc.gpsimd.dma_start(out=xm_sb[:, 1, 3072:4096], in_=x2(1, 3072, 4096))
        bv_ap = bv[:]
        bv_bcast = bass.AP(
            tensor=bv_ap.tensor, offset=bv_ap.offset, ap=[[0, 128]] + list(bv_ap.ap)
        )
        nc.gpsimd.dma_start(out=bv_bc, in_=bv_bcast)

        # ---- projections -------------------------------------------------
        # q is stored twice along partitions (0:64 and 64:128) so energy
        # fills can row-slot-pack two K=64 matmuls into the PE array.
        def colpack_proj(dst_ps, rhs0, rhs1):
            return [
                nc.tensor.matmul(dst_ps[0:64, :], wq_sb[:, 0, :], rhs0,
                                 start=True, stop=False, tile_position=(0, 0)),
                nc.tensor.matmul(dst_ps[64:128, :], wq_sb[:, 0, :], rhs0,
                                 start=True, stop=False, tile_position=(0, 64),
                                 skip_group_check=True),
                nc.tensor.matmul(dst_ps[0:64, :], wq_sb[:, 1, :], rhs1,
                                 start=False, stop=True, tile_position=(0, 0)),
                nc.tensor.matmul(dst_ps[64:128, :], wq_sb[:, 1, :], rhs1,
                                 start=False, stop=True, tile_position=(0, 64),
                                 skip_group_check=True),
            ]

        xqt = [sing.tile([128, 1024], BF16, name=f"xq{i}") for i in range(4)]

        def xk(row, t):
            i, off = (t * NT) // 1024, (t * NT) % 1024
            return xqt[i][row:row + 64, off:off + NT]

        def xq(row, col, w):
            i, cc = col // 1024, col % 1024
            return xqt[i][row:row + 64, cc:cc + w]

        def q_proj(j, prio=0):
            qp = xr_ps.tile([128, 512], F32, tag=f"xr{j % 4}", name=f"qp{j}")
            mms = colpack_proj(qp, xm_sb[:, 0, ts(j, 512)], xm_sb[:, 1, ts(j, 512)])
            for m in mms:
                m.ins.bass_priority = prio
            dst = xqt[j // 2][:, (j % 2) * 512:(j % 2) * 512 + 512]
            # all casts on vector: the scalar queue stays pure exp (any op
            # queued ahead of the first exp delays the whole stream).
            cp = nc.vector.tensor_copy(out=dst, in_=qp)
            cp.ins.bass_priority = -600

        # ---- energy fill / exp plumbing ----------------------------------
        # stream order: tiles 0/1 interleaved at the front so the hi-half
        # (cols 2048:4096) DMA + projection latency hides under low-half
        # exp chunks.
        # stream order: the first eight items are 512-wide sub-chunks of
        # (t, 0) for tiles 0-3 -- the 'a' halves depend ONLY on the first
        # q projection (cols 0:512), so the exp stream starts the moment
        # the first 128KB of x lands, while q1..q7 project underneath.
        chunk_list = [(0, 0, 'a'), (1, 0, 'a'), (2, 0, 'a'), (3, 0, 'a'),
                      (0, 0, 'b'), (1, 0, 'b'), (2, 0, 'b'), (3, 0, 'b'),
                      (0, 1, None), (1, 1, None), (2, 1, None), (3, 1, None),
                      (0, 2, None), (0, 3, None), (1, 2, None), (1, 3, None),
                      (2, 2, None), (2, 3, None), (3, 2, None), (3, 3, None)]
        for t in range(4, NTILES):
            chunk_list += [(t, 0, None), (t, 1, None),
                           (t, 2, None), (t, 3, None)]

        def emit_fill(t, c, sub=None, prio=0, borrow=None):
            m0 = 1024 * c
            if sub == 'a':
                if borrow is not None:
                    # early runway chunks use the (still idle) AV
                    # accumulator banks as extra PSUM so they skip the
                    # 2-slot e-ring WAR entirely.
                    e_t = xr_ps.tile([128, 512], F32, tag=borrow,
                                     name=f"e{t}_{c}a")
                else:
                    e_t = e_ps.tile([128, 512], F32, tag="e", name=f"e{t}_{c}a")
                m1 = nc.tensor.matmul(e_t, xk(0, t), xq(0, m0, 512),
                                      start=True, stop=True,
                                      tile_position=(0, 0))
                m1.ins.bass_priority = prio
                return e_t
            if sub == 'b':
                e_t = e_ps.tile([128, 512], F32, tag="e", name=f"e{t}_{c}b")
                m1 = nc.tensor.matmul(e_t, xk(64, t), xq(64, m0 + 512, 512),
                                      start=True, stop=True,
                                      tile_position=(64, 0),
                                      skip_group_check=True)
                m1.ins.bass_priority = prio
                return e_t
            e_t = e_ps.tile([128, 1024], F32, tag="e", name=f"e{t}_{c}")
            m1 = nc.tensor.matmul(e_t[:, 0:512], xk(0, t), xq(0, m0, 512),
                                  start=True, stop=True, tile_position=(0, 0))
            m2 = nc.tensor.matmul(e_t[:, 512:1024], xk(64, t),
                                  xq(64, m0 + 512, 512),
                                  start=True, stop=True, tile_position=(64, 0),
                                  skip_group_check=True)
            m1.ins.bass_priority = prio
            m2.ins.bass_priority = prio
            return e_t

        # prologue: projections and the first two fills, interleaved so
        # each fill is emitted as soon as its q columns exist.
        q_proj(0, prio=-3000)
        etiles = {(0, 0, 'a'): emit_fill(0, 0, 'a', prio=-2995)}
        q_proj(1, prio=-2990)
        etiles[(1, 0, 'a')] = emit_fill(1, 0, 'a', prio=-2985)
        q_proj(2, prio=-2970)
        q_proj(3, prio=-2960)
        q_proj(4, prio=-2930)
        q_proj(5, prio=-2920)
        q_proj(6, prio=-2910)
        q_proj(7, prio=-2900)

        # per-tile v projections (deprioritized PE gap filler)
        xvt_sb = [
            sing.tile([128, C4], BF16, name=f"xvt{t}") for t in range(NTILES)
        ]
        for t in range(NTILES):
            vp = xr_ps.tile([128, C4], F32, tag=f"xr{t % 4}", name=f"vp{t}")
            half = t // 8
            off = (t % 8) * NT
            mm1 = nc.tensor.matmul(vp, xm_sb[:, 0, ds(half * 1024 + off, NT)],
                                   wv_sb[:, 0, :], start=True, stop=False)
            mm2 = nc.tensor.matmul(vp, xm_sb[:, 1, ds(half * 1024 + off, NT)],
                                   wv_sb[:, 1, :], start=False, stop=True)
            mm1.ins.bass_priority = 500_000 + 2 * t
            mm2.ins.bass_priority = 500_000 + 2 * t + 1
            nc.vector.tensor_add(out=xvt_sb[t], in0=vp, in1=bv_bc)

        # ---- output accumulators (partition-packed: even m-chunk in
        # partitions 0-63, odd in 64-127) -----------------------------------
        xr = [
            xr_ps.tile([128, 512], F32, tag=f"xr{k}", name=f"xr{k}")
            for k in range(4)
        ]

        p_tiles = {}
        xvs_tiles = {}
        rs4_tiles = {}

        def chunk_rowsum(t, c):
            rs4 = rs4_tiles[t]
            p = p_tiles[t]
            last_tile = t == NTILES - 1
            if last_tile and c == 3:
                return  # rowsum came from the exp's accumulator
            if c == 3 or (last_tile and c == 2):
                # direct reduce right after the chunk's exp (off the scalar
                # queue; for the last tile it finishes under the final exp)
                r = nc.vector.tensor_reduce(out=rs4[:, c:c + 1],
                                            in_=p[:, ds(1024 * c, 1024)],
                                            axis=mybir.AxisListType.X, op=ADD)
                if last_tile:
                    r.ins.bass_priority = -500
            else:
                hf = hfp.tile([128, 512], BF16, tag="hf")
                nc.gpsimd.tensor_add(out=hf, in0=p[:, ds(1024 * c, 512)],
                                     in1=p[:, ds(1024 * c + 512, 512)])
                nc.vector.tensor_reduce(out=rs4[:, c:c + 1], in_=hf,
                                        axis=mybir.AxisListType.X, op=ADD)

        def do_exp(t, c, sub):
            p = p_tiles[t]
            e_t = etiles.pop((t, c, sub))
            if t not in rs4_tiles:
                rs4_tiles[t] = small.tile([128, 4], F32, tag="rs4", name=f"rs4_{t}")
            rs4 = rs4_tiles[t]
            last_tile = t == NTILES - 1
            if sub == 'a':
                nc.scalar.activation(out=p[:, ds(1024 * c, 512)],
                                     in_=e_t, func=EXP)
                return
            if sub == 'b':
                nc.scalar.activation(out=p[:, ds(1024 * c + 512, 512)],
                                     in_=e_t, func=EXP)
            elif last_tile and c == 3:
                # the very last exp carries its own rowsum accumulator so
                # the final normalization starts ~300ns after it instead of
                # a 1.2us vector-reduce later.
                nc.scalar.activation(out=p[:, ds(1024 * c, 1024)], in_=e_t,
                                     func=EXP, accum_out=rs4[:, 3:4])
            else:
                nc.scalar.activation(out=p[:, ds(1024 * c, 1024)], in_=e_t,
                                     func=EXP)
            chunk_rowsum(t, c)

        def rowsum_tile(t):
            rs4 = rs4_tiles.pop(t)
            rs = small.tile([128, 1], F32, tag="rs")
            r1 = nc.vector.tensor_reduce(out=rs, in_=rs4,
                                         axis=mybir.AxisListType.X, op=ADD)
            rr = small.tile([128, 1], F32, tag="rr")
            r2 = nc.vector.reciprocal(out=rr, in_=rs)
            xvs = small.tile([128, C4], BF16, tag="xvs")
            r3 = nc.vector.tensor_scalar_mul(out=xvs, in0=xvt_sb[t], scalar1=rr)
            # the normalization chain gates AV(t): never let the scheduler
            # slip a bulk reduce ahead of it on the vector queue.
            for r in (r1, r2, r3):
                r.ins.bass_priority = -500
            xvs_tiles[t] = xvs

        def emit_av_bank(t, k):
            # one bank's worth of AV (4 matmuls): emitted at four separate
            # stream positions so the in-order PE never sees an AV burst
            # longer than ~1us between energy fills.
            p = p_tiles[t]
            xvs = xvs_tiles[t]
            first = t == 0
            last = t == NTILES - 1
            # 512-wide everywhere: AV bursts are already bounded to one
            # bank by the spread emission, and half the matmul/LDWEIGHTS
            # count keeps the PE ahead of the exp stream in HAM-throttled
            # windows.
            av_w = 512
            for j in (2 * k, 2 * k + 1):
                po = (j % 2) * 64
                for s in range(512 // av_w):
                    mm = nc.tensor.matmul(
                        xr[k][po:po + 64, ds(s * av_w, av_w)], xvs,
                        p[:, ds(j * 512 + s * av_w, av_w)],
                        start=first, stop=last, tile_position=(0, po),
                        skip_group_check=True,
                    )
                    if not last:
                        mm.ins.bass_priority = 1_000_000 + t * 100 + j * 4 + s

        def emit_av(t):
            for k in range(4):
                emit_av_bank(t, k)
            xvs_tiles.pop(t)

        # ---- the stream --------------------------------------------------
        # AV(t) is emitted one tile late (at (t+1, 3)) so in the in-order
        # PE queue ALL of tile t+1's fills statically precede AV(t): a late
        # xvs(t) can then never stall the exp stream behind an AV group.
        for i, (t, c, sub) in enumerate(chunk_list):
            if t not in p_tiles:
                p_tiles[t] = work.tile([128, N], BF16, tag="p", name=f"p{t}")
            do_exp(t, c, sub)
            if i + 2 < len(chunk_list):
                nt_, nc_, ns_ = chunk_list[i + 2]
                if (nt_, nc_, ns_) not in etiles:
                    prio = -2950 + i * 5 if i < 8 else 0
                    etiles[(nt_, nc_, ns_)] = emit_fill(nt_, nc_, ns_,
                                                        prio=prio)
            if sub is None and t >= 4 and (t - 1) in xvs_tiles:
                emit_av_bank(t - 1, c)
                if c == 3:
                    xvs_tiles.pop(t - 1)
            if c == 3 and sub is None:
                rowsum_tile(t)
                if t < 4 and t >= 1 and (t - 1) in xvs_tiles:
                    emit_av(t - 1)
                if t == NTILES - 1:
                    emit_av(t)

        # ---- epilogue: per-bank staggered PSUM->SBUF copy + DMA ----------
        # bf16 partials: the host sums the two per-batch partials in fp32;
        # bf16 here halves the output DMA drain and is well inside the
        # error budget (adds ~0.3% to a 0.46% rel err vs 2% tolerance).
        out_sb = sing.tile([128, 4, 512], BF16)
        # copies first (scalar/vector alternating, in bank-closure order),
        # then the DMA issues: scalar's FIFO must not block a later copy
        # behind an earlier bank's DMA issues.
        for k in range(4):
            if k % 2 == 0:
                nc.scalar.copy(out=out_sb[:, k, :], in_=xr[k])
            else:
                nc.vector.tensor_copy(out=out_sb[:, k, :], in_=xr[k])
        # each bank's two 128KB halves go to different rings so both
        # rings start draining at bank 0's closure and finish together.
        for k in range(4):
            nc.sync.dma_start(out=out_p[:, ts(2 * k, 512)],
                              in_=out_sb[0:64, k, :])
            nc.scalar.dma_start(out=out_p[:, ts(2 * k + 1, 512)],
                                in_=out_sb[64:128, k, :])

    nc.compile()
    return nc


_NC_CACHE = None


def _get_nc():
    global _NC_CACHE
    if _NC_CACHE is None:
        _NC_CACHE = build_nc()
    return _NC_CACHE


def make_in_maps(x, W_qk, W_v, b_v):
    bf = ml_dtypes.bfloat16
    x = np.asarray(x, dtype=np.float32)
    W_qk = np.asarray(W_qk, dtype=np.float32)
    W_v = np.asarray(W_v, dtype=np.float32)
    b_v = np.asarray(b_v, dtype=np.float32)
    xbf = np.ascontiguousarray(x).astype(bf)
    wqt = np.ascontiguousarray((W_qk / np.sqrt(FACTOR)).T).astype(bf)
    wvt = np.ascontiguousarray(W_v.T).astype(bf)
    bvb = np.ascontiguousarray(b_v).astype(bf)
    in_maps = []
    for core in range(8):
        b, h = core // 2, core % 2
        xm = xbf[b] if h == 0 else np.ascontiguousarray(
            np.roll(xbf[b], -NH, axis=1))
        in_maps.append({
            "x_m": xm,
            "wq_t": wqt,
            "wv_t": wvt,
            "bv": bvb,
        })
    return in_maps


def kernel(x, W_qk, W_v, b_v, _trace=False):
    from concourse.bass_utils import run_bass_kernel_spmd

    nc = _get_nc()
    in_maps = make_in_maps(x, W_qk, W_v, b_v)
    res = run_bass_kernel_spmd(nc, in_maps, list(range(8)), trace=_trace)
    if _trace:
        print(f"HW exec time: {res.exec_time_ns} ns")
        print(f"mean exec time: {res.mean_exec_time_ns} ns")
    outs = [np.asarray(res.results[i]["out_p"], dtype=np.float32)
            for i in range(8)]
    out = np.stack([
        outs[2 * b] + np.roll(outs[2 * b + 1], NH, axis=1) for b in range(B)
    ])
    return out.astype(np.float32)
